# revision 34
# baseline (speedup 1.0000x reference)
"""GEAR quantized-KV Llama attention decode step on 8 trn2 NeuronCores.

Sharding: tensor-parallel over heads (4 heads/core x 8 cores), all batches on
every core; each core computes a partial wo-product, summed on host.

v2: fp8 codes (exact for 0..15), bf16 weights, packed per-core blobs so every
DMA is large and contiguous; K-score matmuls use fp8 FWL stationary codes;
V matmuls keep head-dim on partitions; broadcasts via ones-matmul.
"""
import os
import sys
import math

sys.path.insert(0, "/opt/trn_rl_repo")
import numpy as np
import ml_dtypes
from contextlib import ExitStack

import concourse.bass as bass
import concourse.mybir as mybir
import concourse.tile as tile
from concourse import bacc, bass_isa
from concourse.bass_utils import run_bass_kernel_spmd
from concourse.masks import make_identity

B, H, D, HID = 4, 32, 128, 4096
SQ, SF, QL = 4096, 63, 1
GS, RANK = 64, 4
THETA = 10000.0
NCORES = 8
HPC = H // NCORES          # heads per core = 4
NP = B * HPC               # (b,h) pairs per core = 16
NCH = SQ // 128            # 32 s-chunks
G = SQ // GS               # 64 groups along seq (K side)
FD = D // GS               # 2 groups along head_dim (V side)
SFP = SF + 1               # 64 full-precision keys incl the new token
DT = mybir.dt
ISQD = 1.0 / math.sqrt(D)
KBW = G + G + 64 + NCH * RANK + RANK   # kblob width = 64+64+64+128+4 = 324
VBW = NCH * FD + 6 * NCH               # vblob width = 64+192 = 256

BF16 = ml_dtypes.bfloat16
FP8 = ml_dtypes.float8_e4m3

_CACHE = {}


def _build():
    nc = bacc.Bacc("TRN2", target_bir_lowering=False)
    f32, bf16, fp8 = DT.float32, DT.bfloat16, DT.float8e4

    hidq = nc.declare_dram_parameter("hidq", [128, NCH * B], bf16, isOutput=False)
    cosin = nc.declare_dram_parameter("cosin", [B, 2 * HPC * D], f32, isOutput=False)
    wqkv = nc.declare_dram_parameter("wqkv", [128, NCH * 3 * HPC * D], bf16, isOutput=False)
    woc = nc.declare_dram_parameter("woc", [128, HPC * HID], bf16, isOutput=False)
    codes = nc.declare_dram_parameter("codes", [NP, 128, 2 * SQ], fp8, isOutput=False)
    kblob = nc.declare_dram_parameter("kblob", [128, NP * KBW], bf16, isOutput=False)
    vblob = nc.declare_dram_parameter("vblob", [128, NP * VBW], bf16, isOutput=False)
    vpm = nc.declare_dram_parameter("vpm", [6, NP * 128], bf16, isOutput=False)
    vfull = nc.declare_dram_parameter("vfull", [SF, NP * 128], bf16, isOutput=False)
    out = nc.declare_dram_parameter("out", [B, HID], f32, isOutput=True)

    AO = mybir.AluOpType
    AF = mybir.ActivationFunctionType

    with tile.TileContext(nc) as tc, ExitStack() as ctx:
        const = ctx.enter_context(tc.tile_pool(name="const", bufs=1))
        pw = ctx.enter_context(tc.tile_pool(name="pw", bufs=2))
        ictx = ctx.enter_context(ExitStack())
        psml = ictx.enter_context(tc.tile_pool(name="psml", bufs=4))
        pkc = ictx.enter_context(tc.tile_pool(name="pkc", bufs=4))
        psA = ictx.enter_context(tc.tile_pool(name="psA", bufs=4, space="PSUM"))
        pctx = ExitStack()
        psC = pctx.enter_context(tc.tile_pool(name="psC", bufs=1, space="PSUM"))

        # ---- constants / resident blobs ----
        id4 = const.tile([4, 4], f32)
        make_identity(nc, id4[:])
        ones_row = const.tile([1, 128], bf16)
        nc.vector.memset(ones_row[:], 1.0)
        ones_col = const.tile([128, 1], f32)
        nc.vector.memset(ones_col[:], 1.0)

        hid_sb = const.tile([128, NCH, B], bf16)
        nc.sync.dma_start(out=hid_sb[:], in_=hidq[:].rearrange("p (c b) -> p c b", b=B))
        cos_sb = const.tile([B, HPC * D], f32)
        nc.sync.dma_start(out=cos_sb[:], in_=cosin[:, 0:HPC * D])
        sin_sb = const.tile([B, HPC * D], f32)
        nc.sync.dma_start(out=sin_sb[:], in_=cosin[:, HPC * D:2 * HPC * D])
        kblob_sb = const.tile([128, NP, KBW], bf16)
        nc.sync.dma_start(out=kblob_sb[:], in_=kblob[:].rearrange("p (n w) -> p n w", w=KBW))
        vblob_sb = const.tile([128, NP, VBW], bf16)
        nc.sync.dma_start(out=vblob_sb[:], in_=vblob[:].rearrange("p (n w) -> p n w", w=VBW))
        vpm_sb = const.tile([6, NP, 128], bf16)
        nc.sync.dma_start(out=vpm_sb[:], in_=vpm[:].rearrange("p (n d) -> p n d", d=128))
        vfull_sb = const.tile([SFP, NP, 128], bf16)
        nc.sync.dma_start(out=vfull_sb[0:SF], in_=vfull[:].rearrange("p (n d) -> p n d", d=128))
        wo_sb = const.tile([128, HPC, HID], bf16)

        # ---- projections: psum[b, 1536] = sum_c hid_c^T @ wqkv_c ----
        pps = psC.tile([B, 3 * HPC * D], f32, tag="proj")
        for blk in range(4):
            slab = pw.tile([128, 8, 3 * HPC * D], bf16, tag="wslab")
            nc.scalar.dma_start(
                out=slab[:],
                in_=wqkv[:, 8 * blk * 1536:(8 * blk + 8) * 1536].rearrange(
                    "p (c n) -> p c n", n=1536),
            )
            for j in range(8):
                c = 8 * blk + j
                for nb in range(3):
                    nc.tensor.matmul(pps[:, nb * 512:(nb + 1) * 512],
                                     hid_sb[:, c, :], slab[:, j, nb * 512:(nb + 1) * 512],
                                     start=(c == 0), stop=(c == 31))
        qkv_sb = const.tile([B, 3 * HPC * D], f32)
        nc.scalar.copy(qkv_sb[:], pps[:])
        q_sb = qkv_sb[:, 0:512]
        k_sb = qkv_sb[:, 512:1024]

        # ---- RoPE on q and k (rows [B, HPC*D], f32) ----
        def rope(x_v, tagp):
            rot = const.tile([B, HPC * D], f32, tag=tagp + "rot")
            xv = x_v.rearrange("b (h two d) -> b h two d", two=2, d=64)
            rv = rot[:].rearrange("b (h two d) -> b h two d", two=2, d=64)
            nc.vector.tensor_scalar(rv[:, :, 0, :], xv[:, :, 1, :], -1.0, None, AO.mult)
            nc.vector.tensor_copy(rv[:, :, 1, :], xv[:, :, 0, :])
            nc.vector.tensor_tensor(rot[:], rot[:], sin_sb[:], AO.mult)
            ro = const.tile([B, HPC * D], f32, tag=tagp + "ro")
            nc.vector.tensor_tensor(ro[:], x_v, cos_sb[:], AO.mult)
            nc.vector.tensor_tensor(ro[:], ro[:], rot[:], AO.add)
            return ro
        qro = rope(q_sb, "q")
        kro = rope(k_sb, "k")

        # per-head transposed columns: qscT [128, h, b] (scaled by 1/sqrt(D)), kT
        qscT = const.tile([128, HPC, B], bf16)
        qsc32p = const.tile([128, NP], f32)       # pair-major (b*HPC+h) f32 copy
        qsc32v = qsc32p[:].rearrange("p (b h) -> p h b", h=HPC)
        kT = const.tile([128, HPC, B], bf16)
        for h in range(HPC):
            pq = psC.tile([128, B], f32, tag="tr")
            nc.tensor.transpose(pq[:], qro[0:B, h * D:(h + 1) * D], id4[:])
            nc.scalar.mul(qscT[:, h, :], pq[:], ISQD)
            nc.scalar.mul(qsc32v[:, h, :], pq[:], ISQD)
            pk = psC.tile([128, B], f32, tag="tr")
            nc.tensor.transpose(pk[:], kro[0:B, h * D:(h + 1) * D], id4[:])
            nc.scalar.copy(kT[:, h, :], pk[:])
        pctx.close()

        # new-token k columns for all pairs (one strided copy), v rows (cast DMAs
        # issued here so they run on the idle gpsimd queue during projections)
        nc.vector.tensor_copy(
            kblob_sb[:, :, 195:196].rearrange("p (b h) one -> p b (h one)", h=HPC),
            kT[:].rearrange("p h b -> p b h"))
        for p in range(NP):
            b, h = p // HPC, p % HPC
            nc.gpsimd.dma_start(
                out=vfull_sb[SF:SFP, p, :],
                in_=qkv_sb[b:b + 1, 1024 + h * D:1024 + (h + 1) * D])
        # qs for all pairs in one op: qs_all[p, pair, g] = kscale * q / sqrt(D)
        qs_all = const.tile([128, NP, G], bf16)
        nc.vector.tensor_tensor(qs_all[:], kblob_sb[:, :, 0:G],
                                qsc32p[:, :, None].to_broadcast((128, NP, G)), AO.mult)

        woin = const.tile([128, NP], bf16)

        # ---- per (b, h) attention, software-pipelined ----
        # Round r emits: B1(r-2) softmax-dependent DVE/gpsimd prep, A(r) DMA +
        # K-score matmuls + logit assembly, psr(r-2) matmul, M(r-1) exp chain,
        # B2(r-2) V matmuls + normalize + combine. Normalization is deferred to
        # the woin copies so nothing upstream waits on the exp-sum reduction.
        # pp layout: psk 0:64 | psv 64:66 | att 66:99 | qrow 99:167 |
        #            qrb 167:171 | psr 171:172
        st = {}

        def stage_a(p):
            b, h = p // HPC, p % HPC
            qcol = qscT[:, h, b:b + 1]
            if p == 4:
                # wo weights stream during attention, on the ACT ring
                nc.scalar.dma_start(out=wo_sb[:],
                                    in_=woc[:].rearrange("p (h n) -> p h n", n=HID))

            cds = pkc.tile([128, 2, SQ], fp8, tag="codes")
            nc.sync.dma_start(out=cds[:], in_=codes[p].rearrange(
                "p (two s) -> p two s", s=SQ))
            kc = cds[:, 0, :]

            kmnq = kblob_sb[:, p, G:G + 68]            # kmn | keyq
            kfp = kblob_sb[:, p, 132:132 + SFP]
            kp_v = kblob_sb[:, p, 196:196 + NCH * RANK].rearrange(
                "p (c r) -> p c r", r=RANK)

            pp = psA.tile([128, 172], f32, tag="pp")
            psk = pp[:, 0:64]

            # kf scores straight into the logit column [64, 98:99]
            nc.tensor.matmul(pp[0:SFP, 98:99], kfp, qcol, start=True, stop=True)
            # mn bias row [1, 99:163], qr row [1, 163:167]
            nc.tensor.matmul(pp[0:1, 99:167], qcol, kmnq, start=True, stop=True)
            qb_row = psml.tile([1, 68], bf16, tag="qbrow")
            nc.scalar.copy(qb_row[:], pp[0:1, 99:167])
            # bias broadcast seeds psk; chunk matmuls accumulate onto it.
            # qr broadcast to [128, 167:171]
            nc.tensor.matmul(psk, ones_row[:], qb_row[0:1, 0:64],
                             start=True, stop=False, skip_group_check=True)
            nc.tensor.matmul(pp[:, 167:171], ones_row[:], qb_row[0:1, 64:68],
                             start=True, stop=True)

            # quant K scores: psk[s, 2c + g'] += codes^T qs per chunk
            for c in range(NCH):
                nc.tensor.matmul(psk[:, 2 * c:2 * c + 2], kc[:, c * 128:(c + 1) * 128],
                                 qs_all[:, p, 2 * c:2 * c + 2], start=False,
                                 stop=(c == NCH - 1), skip_group_check=True)

            # low-rank correction lr[s, c] = sum_r kp[s,c,r] * qr[r]
            lrt = psml.tile([128, NCH, RANK], bf16, tag="lrt")
            nc.vector.tensor_tensor(lrt[:], kp_v,
                                    pp[:, None, 167:171].to_broadcast((128, NCH, RANK)),
                                    AO.mult)
            lr = psml.tile([128, NCH], f32, tag="lr")
            nc.vector.reduce_sum(lr[:], lrt[:], axis=mybir.AxisListType.X)

            # logits att = pp[:, 66:99]: quant scores + lr, kf col, -inf filler
            pskv = psk.rearrange("p (c two) -> p c two", two=2)
            nc.vector.tensor_tensor(pp[0:64, 66:98], pskv[0:64, :, 0], lr[0:64, :], AO.add)
            nc.vector.tensor_tensor(pp[64:128, 66:98], pskv[64:128, :, 1], lr[64:128, :], AO.add)
            nc.vector.memset(pp[64:128, 98:99], -1e9)

            m1 = psml.tile([128, 1], f32, tag="m1")
            nc.vector.reduce_max(m1[:], pp[:, 66:99], axis=mybir.AxisListType.X)
            mg = psml.tile([128, 1], f32, tag="mg")
            nc.gpsimd.partition_all_reduce(mg[:], m1[:], 128, bass_isa.ReduceOp.max)
            st[p] = [cds, pp, mg]

        def stage_m(p):
            cds, pp, mg = st[p]
            negm = psml.tile([128, 1], f32, tag="negm")
            nc.vector.tensor_scalar(negm[:], mg[:], -1.0, None, AO.mult)
            e = psml.tile([128, NCH + 1], bf16, tag="e")
            ssum = psml.tile([128, 1], f32, tag="ssum")
            nc.scalar.activation(e[:], pp[:, 66:99], AF.Exp, bias=negm[:, 0:1],
                                 scale=1.0, alpha=0.0, accum_out=ssum[:])
            sg = psml.tile([128, 1], f32, tag="sg")
            nc.gpsimd.partition_all_reduce(sg[:], ssum[:], 128, bass_isa.ReduceOp.add)
            st[p] = [cds, pp, e, sg]

        def stage_b1(p):
            cds, pp, e, sg = st[p]
            vsc = vblob_sb[:, p, 0:NCH * FD].rearrange("p (c g) -> p c g", g=FD)
            vqmn = vblob_sb[:, p, NCH * FD:VBW].rearrange("p (j c) -> p j c", c=NCH)

            # unnormalized moving cols: e * vscale per chunk (DVE)
            ev = e[:, 0:NCH, None]
            awvs = psml.tile([128, NCH, FD], bf16, tag="awvs")
            nc.vector.tensor_tensor(awvs[:], ev.to_broadcast((128, NCH, FD)), vsc,
                                    AO.mult)
            awf2 = psml.tile([SFP, FD], bf16, tag="awf")
            nc.vector.tensor_copy(awf2[:], e[0:SFP, NCH:NCH + 1].to_broadcast((SFP, FD)))
            # vq/vmn partials on gpsimd (keeps DVE free)
            prod6 = psml.tile([128, 6, NCH], bf16, tag="prod6")
            nc.gpsimd.tensor_tensor(prod6[:], vqmn,
                                    e[:, None, 0:NCH].to_broadcast((128, 6, NCH)),
                                    AO.mult)
            part6 = psml.tile([128, 6], f32, tag="part6")
            nc.vector.reduce_sum(part6[:], prod6[:], axis=mybir.AxisListType.X)
            st[p] += [awvs, awf2, part6]

        def stage_psr(p):
            part6 = st[p][6]
            pp = st[p][1]
            nc.tensor.matmul(pp[0:6, 171:172], part6[:], ones_col[:],
                             start=True, stop=True)

        def stage_b2(p):
            idx = (p % HPC) * B + p // HPC
            cds, pp, e, sg, awvs, awf2, part6 = st.pop(p)
            vt = cds[:, 1, :].rearrange("p (c d) -> p c d", d=128)
            psv = pp[:, 64:66]
            rvec2 = psml.tile([6, FD], bf16, tag="rvec")
            nc.vector.tensor_copy(rvec2[:], pp[0:6, 171:172].to_broadcast((6, FD)))

            # V matmuls: quant chunks + residual + low-rank + mn corrections,
            # all accumulated into psv[d, 0:2] (corrections apply to both cols)
            for c in range(NCH):
                nc.tensor.matmul(psv, vt[:, c, :], awvs[:, c, :],
                                 start=(c == 0), stop=False)
            nc.tensor.matmul(psv, vfull_sb[:, p, :], awf2[:], start=False, stop=False)
            nc.tensor.matmul(psv, vpm_sb[:, p, :], rvec2[:], start=False, stop=True)

            recip = psml.tile([128, 1], f32, tag="recip")
            nc.vector.reciprocal(recip[:], sg[:])
            nc.vector.tensor_scalar(woin[0:64, idx:idx + 1], psv[0:64, 0:1],
                                    recip[0:64, 0:1], None, AO.mult)
            nc.vector.tensor_scalar(woin[64:128, idx:idx + 1], psv[64:128, 1:2],
                                    recip[64:128, 0:1], None, AO.mult)

        for r in range(NP + 2):
            if r >= 2:
                stage_b1(r - 2)
            if r < NP:
                stage_a(r)
            if r >= 2:
                stage_psr(r - 2)
            if r >= 1 and r - 1 < NP:
                stage_m(r - 1)
            if r >= 2:
                stage_b2(r - 2)

        # ---- tail: wo matmul ----
        ictx.close()
        psO = ctx.enter_context(tc.tile_pool(name="psO", bufs=2, space="PSUM"))
        for half in range(2):
            po = psO.tile([B, HID // 2], f32, tag="po")
            for h in range(HPC):
                for nb in range(4):
                    j0 = half * 2048 + nb * 512
                    nc.tensor.matmul(po[:, nb * 512:(nb + 1) * 512],
                                     woin[:, h * B:(h + 1) * B], wo_sb[:, h, j0:j0 + 512],
                                     start=(h == 0), stop=(h == HPC - 1))
            osb = const.tile([B, HID // 2], f32, tag=f"osb{half}")
            nc.scalar.copy(osb[:], po[:])
            nc.sync.dma_start(out=out[:, half * 2048:(half + 1) * 2048], in_=osb[:])

    nc.compile()
    return nc


def _host_prep(inputs):
    hs = np.asarray(inputs["hidden_states"], np.float32)
    pos = np.asarray(inputs["position_ids"])
    inv = 1.0 / (THETA ** (np.arange(0, D, 2, dtype=np.float32) / D))
    fr = pos[:, 0].astype(np.float32)[:, None] * inv[None, :]
    emb = np.concatenate([fr, fr], axis=1)
    cos_b = np.cos(emb).astype(np.float32)
    sin_b = np.sin(emb).astype(np.float32)
    cosin = np.ascontiguousarray(
        np.concatenate([np.tile(cos_b, (1, HPC)), np.tile(sin_b, (1, HPC))], axis=1))
    # hidq [128, NCH*B] bf16
    hidq = np.ascontiguousarray(
        hs[:, 0, :].T.reshape(NCH, 128, B).transpose(1, 0, 2).reshape(128, NCH * B)
    ).astype(BF16)

    wq, wk, wv, wo = (np.asarray(inputs[k], np.float32) for k in ("wq", "wk", "wv", "wo"))
    kq_f = np.asarray(inputs["k_quant"])
    vq_f = np.asarray(inputs["v_quant"])
    ks_f = np.asarray(inputs["k_scale"], np.float32)
    km_f = np.asarray(inputs["k_mn"], np.float32)
    kf_f = np.asarray(inputs["k_full"], np.float32)
    kp_f = np.asarray(inputs["key_p"], np.float32)
    kqr_f = np.asarray(inputs["key_q"], np.float32)
    vs_f = np.asarray(inputs["v_scale"], np.float32)
    vm_f = np.asarray(inputs["v_mn"], np.float32)
    vf_f = np.asarray(inputs["v_full"], np.float32)
    vqv_f = np.asarray(inputs["value_q"], np.float32)
    vp_f = np.asarray(inputs["value_p"], np.float32)

    in_maps = []
    for core in range(NCORES):
        h0 = core * HPC
        sl = slice(h0 * D, (h0 + HPC) * D)
        hsl = slice(h0, h0 + HPC)

        wqkv = np.concatenate([wq[sl].T, wk[sl].T, wv[sl].T], axis=1)  # [4096, 1536]
        wqkv = wqkv.reshape(NCH, 128, 3 * HPC * D).transpose(1, 0, 2).reshape(128, -1)
        woc = wo[:, sl].T.reshape(HPC, 128, HID).transpose(1, 0, 2).reshape(128, -1)

        codes = np.empty((NP, 128, 2 * SQ), FP8)
        codes[:, :, 0:SQ] = kq_f[:, hsl].reshape(NP, 128, SQ).astype(FP8)
        # vcode: [B,HPC,SQ,D] -> [pair, p=s%128, c, d]
        codes[:, :, SQ:2 * SQ] = (vq_f[:, hsl].reshape(B, HPC, NCH, 128, D)
                                  .transpose(0, 1, 3, 2, 4).reshape(NP, 128, NCH * 128)
                                  .astype(FP8))

        kblob = np.zeros((128, NP, KBW), np.float32)
        kblob[:, :, 0:G] = ks_f[:, hsl].reshape(NP, 128, G).transpose(1, 0, 2)
        kblob[:, :, G:2 * G] = km_f[:, hsl].reshape(NP, 128, G).transpose(1, 0, 2)
        kblob[:, :, 128:132] = kqr_f[:, hsl].reshape(NP, 128, RANK).transpose(1, 0, 2)
        kblob[:, :, 132:132 + SF] = kf_f[:, hsl].reshape(NP, SF, 128).transpose(2, 0, 1)
        kblob[:, :, 196:196 + NCH * RANK] = (
            kp_f[:, hsl].reshape(B, HPC, NCH, 128, RANK)
            .transpose(3, 0, 1, 2, 4).reshape(128, NP, NCH * RANK))

        vblob = np.zeros((128, NP, VBW), np.float32)
        vblob[:, :, 0:NCH * FD] = (
            vs_f[:, hsl].reshape(B, HPC, NCH, 128, FD)
            .transpose(3, 0, 1, 2, 4).reshape(128, NP, NCH * FD))
        vblob[:, :, NCH * FD:NCH * FD + 4 * NCH] = (
            vqv_f[:, hsl].reshape(B, HPC, NCH, 128, RANK)
            .transpose(3, 0, 1, 4, 2).reshape(128, NP, 4 * NCH))
        vblob[:, :, NCH * FD + 4 * NCH:VBW] = (
            vm_f[:, hsl].reshape(B, HPC, NCH, 128, FD)
            .transpose(3, 0, 1, 4, 2).reshape(128, NP, 2 * NCH))

        vpm = np.zeros((6, NP, 128), np.float32)
        vpm[0:4] = vp_f[:, hsl].reshape(NP, 128, RANK).transpose(2, 0, 1)
        vpm[4, :, 0:64] = 1.0
        vpm[5, :, 64:128] = 1.0

        vfull = vf_f[:, hsl].reshape(NP, SF, 128).transpose(1, 0, 2)

        m = {
            "hidq": hidq, "cosin": cosin,
            "wqkv": np.ascontiguousarray(wqkv).astype(BF16),
            "woc": np.ascontiguousarray(woc).astype(BF16),
            "codes": codes,
            "kblob": kblob.reshape(128, NP * KBW).astype(BF16),
            "vblob": vblob.reshape(128, NP * VBW).astype(BF16),
            "vpm": vpm.reshape(6, NP * 128).astype(BF16),
            "vfull": np.ascontiguousarray(vfull).reshape(SF, NP * 128).astype(BF16),
        }
        in_maps.append(m)
    return in_maps


def kernel(**inputs):
    if "nc" not in _CACHE:
        _CACHE["nc"] = _build()
    nc = _CACHE["nc"]
    in_maps = _host_prep(inputs)
    res = run_bass_kernel_spmd(nc, in_maps, list(range(NCORES)),
                               trace=bool(os.environ.get("K_TRACE")))
    kernel.last = res
    total = np.zeros((B, HID), np.float32)
    for r in res.results:
        total += r["out"]
    return total.reshape(B, QL, HID)


# revision 35
# speedup vs baseline: 2.2271x; 2.2271x over previous
"""GEAR quantized-KV Llama attention decode step on 8 trn2 NeuronCores.

Sharding: tensor-parallel over heads (4 heads/core x 8 cores), all batches on
every core; each core computes a partial wo-product, summed on host.

v2: fp8 codes (exact for 0..15), bf16 weights, packed per-core blobs so every
DMA is large and contiguous; K-score matmuls use fp8 FWL stationary codes;
V matmuls keep head-dim on partitions; broadcasts via ones-matmul.
"""
import os
import sys
import math

sys.path.insert(0, "/opt/trn_rl_repo")
import numpy as np
import ml_dtypes
from contextlib import ExitStack

import concourse.bass as bass
import concourse.mybir as mybir
import concourse.tile as tile
from concourse import bacc, bass_isa
from concourse.bass_utils import run_bass_kernel_spmd
from concourse.masks import make_identity

B, H, D, HID = 4, 32, 128, 4096
SQ, SF, QL = 4096, 63, 1
GS, RANK = 64, 4
THETA = 10000.0
NCORES = 8
HPC = H // NCORES          # heads per core = 4
NP = B * HPC               # (b,h) pairs per core = 16
NCH = SQ // 128            # 32 s-chunks
G = SQ // GS               # 64 groups along seq (K side)
FD = D // GS               # 2 groups along head_dim (V side)
SFP = SF + 1               # 64 full-precision keys incl the new token
DT = mybir.dt
ISQD = 1.0 / math.sqrt(D)
KBW = G + G + 64 + NCH * RANK + RANK   # kblob width = 64+64+64+128+4 = 324
VBW = NCH * FD + 6 * NCH               # vblob width = 64+192 = 256

BF16 = ml_dtypes.bfloat16
FP8 = ml_dtypes.float8_e4m3

_CACHE = {}


def _build():
    nc = bacc.Bacc("TRN2", target_bir_lowering=False)
    f32, bf16, fp8 = DT.float32, DT.bfloat16, DT.float8e4

    hidq = nc.declare_dram_parameter("hidq", [128, NCH * B], bf16, isOutput=False)
    cosin = nc.declare_dram_parameter("cosin", [B, 2 * HPC * D], f32, isOutput=False)
    wqkv = nc.declare_dram_parameter("wqkv", [128, NCH * 3 * HPC * D], bf16, isOutput=False)
    woc = nc.declare_dram_parameter("woc", [128, HPC * HID], bf16, isOutput=False)
    codes = nc.declare_dram_parameter("codes", [NP, 128, 2 * SQ], fp8, isOutput=False)
    kblob = nc.declare_dram_parameter("kblob", [128, NP * KBW], bf16, isOutput=False)
    vblob = nc.declare_dram_parameter("vblob", [128, NP * VBW], bf16, isOutput=False)
    vpm = nc.declare_dram_parameter("vpm", [6, NP * 128], bf16, isOutput=False)
    vfull = nc.declare_dram_parameter("vfull", [SF, NP * 128], bf16, isOutput=False)
    out = nc.declare_dram_parameter("out", [B, HID], f32, isOutput=True)

    AO = mybir.AluOpType
    AF = mybir.ActivationFunctionType

    with tile.TileContext(nc) as tc, ExitStack() as ctx:
        const = ctx.enter_context(tc.tile_pool(name="const", bufs=1))
        pw = ctx.enter_context(tc.tile_pool(name="pw", bufs=2))
        ictx = ctx.enter_context(ExitStack())
        psml = ictx.enter_context(tc.tile_pool(name="psml", bufs=4))
        pkc = ictx.enter_context(tc.tile_pool(name="pkc", bufs=4))
        psA = ictx.enter_context(tc.tile_pool(name="psA", bufs=4, space="PSUM"))
        pctx = ExitStack()
        psC = pctx.enter_context(tc.tile_pool(name="psC", bufs=1, space="PSUM"))

        # ---- constants / resident blobs ----
        id4 = const.tile([4, 4], f32)
        make_identity(nc, id4[:])
        ones_row = const.tile([1, 128], bf16)
        nc.vector.memset(ones_row[:], 1.0)
        ones_col = const.tile([128, 1], f32)
        nc.vector.memset(ones_col[:], 1.0)

        hid_sb = const.tile([128, NCH, B], bf16)
        nc.sync.dma_start(out=hid_sb[:], in_=hidq[:].rearrange("p (c b) -> p c b", b=B))
        cos_sb = const.tile([B, HPC * D], f32)
        nc.sync.dma_start(out=cos_sb[:], in_=cosin[:, 0:HPC * D])
        sin_sb = const.tile([B, HPC * D], f32)
        nc.sync.dma_start(out=sin_sb[:], in_=cosin[:, HPC * D:2 * HPC * D])
        kblob_sb = const.tile([128, NP, KBW], bf16)
        nc.sync.dma_start(out=kblob_sb[:], in_=kblob[:].rearrange("p (n w) -> p n w", w=KBW))
        vblob_sb = const.tile([128, NP, VBW], bf16)
        nc.sync.dma_start(out=vblob_sb[:], in_=vblob[:].rearrange("p (n w) -> p n w", w=VBW))
        vpm_sb = const.tile([6, NP, 128], bf16)
        nc.sync.dma_start(out=vpm_sb[:], in_=vpm[:].rearrange("p (n d) -> p n d", d=128))
        vfull_sb = const.tile([SFP, NP, 128], bf16)
        nc.sync.dma_start(out=vfull_sb[0:SF], in_=vfull[:].rearrange("p (n d) -> p n d", d=128))
        wo_sb = const.tile([128, HPC, HID], bf16)

        # ---- projections: psum[b, 1536] = sum_c hid_c^T @ wqkv_c ----
        pps = psC.tile([B, 3 * HPC * D], f32, tag="proj")
        for blk in range(4):
            slab = pw.tile([128, 8, 3 * HPC * D], bf16, tag="wslab")
            nc.scalar.dma_start(
                out=slab[:],
                in_=wqkv[:, 8 * blk * 1536:(8 * blk + 8) * 1536].rearrange(
                    "p (c n) -> p c n", n=1536),
            )
            for j in range(8):
                c = 8 * blk + j
                for nb in range(3):
                    nc.tensor.matmul(pps[:, nb * 512:(nb + 1) * 512],
                                     hid_sb[:, c, :], slab[:, j, nb * 512:(nb + 1) * 512],
                                     start=(c == 0), stop=(c == 31))
        qkv_sb = const.tile([B, 3 * HPC * D], f32)
        nc.scalar.copy(qkv_sb[:], pps[:])
        q_sb = qkv_sb[:, 0:512]
        k_sb = qkv_sb[:, 512:1024]

        # ---- RoPE on q and k (rows [B, HPC*D], f32) ----
        def rope(x_v, tagp):
            rot = const.tile([B, HPC * D], f32, tag=tagp + "rot")
            xv = x_v.rearrange("b (h two d) -> b h two d", two=2, d=64)
            rv = rot[:].rearrange("b (h two d) -> b h two d", two=2, d=64)
            nc.vector.tensor_scalar(rv[:, :, 0, :], xv[:, :, 1, :], -1.0, None, AO.mult)
            nc.vector.tensor_copy(rv[:, :, 1, :], xv[:, :, 0, :])
            nc.vector.tensor_tensor(rot[:], rot[:], sin_sb[:], AO.mult)
            ro = const.tile([B, HPC * D], f32, tag=tagp + "ro")
            nc.vector.tensor_tensor(ro[:], x_v, cos_sb[:], AO.mult)
            nc.vector.tensor_tensor(ro[:], ro[:], rot[:], AO.add)
            return ro
        qro = rope(q_sb, "q")
        kro = rope(k_sb, "k")

        # per-head transposed columns: qscT [128, h, b] (scaled by 1/sqrt(D)), kT
        qscT = const.tile([128, HPC, B], bf16)
        qsc32p = const.tile([128, NP], f32)       # pair-major (b*HPC+h) f32 copy
        qsc32v = qsc32p[:].rearrange("p (b h) -> p h b", h=HPC)
        kT = const.tile([128, HPC, B], bf16)
        for h in range(HPC):
            pq = psC.tile([128, B], f32, tag="tr")
            nc.tensor.transpose(pq[:], qro[0:B, h * D:(h + 1) * D], id4[:])
            nc.scalar.mul(qscT[:, h, :], pq[:], ISQD)
            nc.scalar.mul(qsc32v[:, h, :], pq[:], ISQD)
            pk = psC.tile([128, B], f32, tag="tr")
            nc.tensor.transpose(pk[:], kro[0:B, h * D:(h + 1) * D], id4[:])
            nc.scalar.copy(kT[:, h, :], pk[:])
        pctx.close()

        # new-token k columns for all pairs (one strided copy), v rows (cast DMAs
        # issued here so they run on the idle gpsimd queue during projections)
        nc.vector.tensor_copy(
            kblob_sb[:, :, 195:196].rearrange("p (b h) one -> p b (h one)", h=HPC),
            kT[:].rearrange("p h b -> p b h"))
        for p in range(NP):
            b, h = p // HPC, p % HPC
            nc.gpsimd.dma_start(
                out=vfull_sb[SF:SFP, p, :],
                in_=qkv_sb[b:b + 1, 1024 + h * D:1024 + (h + 1) * D])
        # qs for all pairs in one op: qs_all[p, pair, g] = kscale * q / sqrt(D)
        qs_all = const.tile([128, NP, G], bf16)
        nc.vector.tensor_tensor(qs_all[:], kblob_sb[:, :, 0:G],
                                qsc32p[:, :, None].to_broadcast((128, NP, G)), AO.mult)

        woin = const.tile([128, NP], bf16)

        # ---- per (b, h) attention, software-pipelined ----
        # Round r emits: B1(r-2) softmax-dependent DVE/gpsimd prep, A(r) DMA +
        # K-score matmuls + logit assembly, psr(r-2) matmul, M(r-1) exp chain,
        # B2(r-2) V matmuls + normalize + combine. Normalization is deferred to
        # the woin copies so nothing upstream waits on the exp-sum reduction.
        # pp layout: psk 0:64 | psv 64:66 | att 66:99 | qrow 99:167 |
        #            qrb 167:171 | psr 171:172
        st = {}

        def stage_a(p):
            b, h = p // HPC, p % HPC
            qcol = qscT[:, h, b:b + 1]
            if p == 4:
                # wo weights stream during attention, on the ACT ring
                nc.scalar.dma_start(out=wo_sb[:],
                                    in_=woc[:].rearrange("p (h n) -> p h n", n=HID))

            cds = pkc.tile([128, 2, SQ], fp8, tag="codes")
            nc.sync.dma_start(out=cds[:], in_=codes[p].rearrange(
                "p (two s) -> p two s", s=SQ))
            kc = cds[:, 0, :]

            kmnq = kblob_sb[:, p, G:G + 68]            # kmn | keyq
            kfp = kblob_sb[:, p, 132:132 + SFP]
            kp_v = kblob_sb[:, p, 196:196 + NCH * RANK].rearrange(
                "p (c r) -> p c r", r=RANK)

            pp = psA.tile([128, 172], f32, tag="pp")
            psk = pp[:, 0:64]

            # kf scores straight into the logit column [64, 98:99]
            nc.tensor.matmul(pp[0:SFP, 98:99], kfp, qcol, start=True, stop=True)
            # mn bias row [1, 99:163], qr row [1, 163:167]
            nc.tensor.matmul(pp[0:1, 99:167], qcol, kmnq, start=True, stop=True)
            qb_row = psml.tile([1, 68], bf16, tag="qbrow")
            nc.scalar.copy(qb_row[:], pp[0:1, 99:167])
            # bias broadcast seeds psk; chunk matmuls accumulate onto it.
            # qr broadcast to [128, 167:171]
            nc.tensor.matmul(psk, ones_row[:], qb_row[0:1, 0:64],
                             start=True, stop=False, skip_group_check=True)
            nc.tensor.matmul(pp[:, 167:171], ones_row[:], qb_row[0:1, 64:68],
                             start=True, stop=True)

            # quant K scores: psk[s, 2c + g'] += codes^T qs per chunk
            for c in range(NCH):
                nc.tensor.matmul(psk[:, 2 * c:2 * c + 2], kc[:, c * 128:(c + 1) * 128],
                                 qs_all[:, p, 2 * c:2 * c + 2], start=False,
                                 stop=(c == NCH - 1), skip_group_check=True)

            # low-rank correction lr[s, c] = sum_r kp[s,c,r] * qr[r]
            lrt = psml.tile([128, NCH, RANK], bf16, tag="lrt")
            nc.vector.tensor_tensor(lrt[:], kp_v,
                                    pp[:, None, 167:171].to_broadcast((128, NCH, RANK)),
                                    AO.mult)
            lr = psml.tile([128, NCH], f32, tag="lr")
            nc.vector.reduce_sum(lr[:], lrt[:], axis=mybir.AxisListType.X)

            # logits att = pp[:, 66:99]: quant scores + lr, kf col, -inf filler
            pskv = psk.rearrange("p (c two) -> p c two", two=2)
            nc.vector.tensor_tensor(pp[0:64, 66:98], pskv[0:64, :, 0], lr[0:64, :], AO.add)
            nc.vector.tensor_tensor(pp[64:128, 66:98], pskv[64:128, :, 1], lr[64:128, :], AO.add)
            nc.vector.memset(pp[64:128, 98:99], -1e9)

            m1 = psml.tile([128, 1], f32, tag="m1")
            nc.vector.reduce_max(m1[:], pp[:, 66:99], axis=mybir.AxisListType.X)
            mg = psml.tile([128, 1], f32, tag="mg")
            nc.gpsimd.partition_all_reduce(mg[:], m1[:], 128, bass_isa.ReduceOp.max)
            st[p] = [cds, pp, mg]

        def stage_m(p):
            cds, pp, mg = st[p]
            negm = psml.tile([128, 1], f32, tag="negm")
            nc.vector.tensor_scalar(negm[:], mg[:], -1.0, None, AO.mult)
            e = psml.tile([128, NCH + 1], bf16, tag="e")
            ssum = psml.tile([128, 1], f32, tag="ssum")
            nc.scalar.activation(e[:], pp[:, 66:99], AF.Exp, bias=negm[:, 0:1],
                                 scale=1.0, alpha=0.0, accum_out=ssum[:])
            sg = psml.tile([128, 1], f32, tag="sg")
            nc.gpsimd.partition_all_reduce(sg[:], ssum[:], 128, bass_isa.ReduceOp.add)
            st[p] = [cds, pp, e, sg]

        def stage_b1(p):
            cds, pp, e, sg = st[p]
            vsc = vblob_sb[:, p, 0:NCH * FD].rearrange("p (c g) -> p c g", g=FD)
            vqmn = vblob_sb[:, p, NCH * FD:VBW].rearrange("p (j c) -> p j c", c=NCH)

            # unnormalized moving cols: e * vscale per chunk (DVE)
            ev = e[:, 0:NCH, None]
            awvs = psml.tile([128, NCH, FD], bf16, tag="awvs")
            nc.vector.tensor_tensor(awvs[:], ev.to_broadcast((128, NCH, FD)), vsc,
                                    AO.mult)
            awf2 = psml.tile([SFP, FD], bf16, tag="awf")
            nc.vector.tensor_copy(awf2[:], e[0:SFP, NCH:NCH + 1].to_broadcast((SFP, FD)))
            prod6 = psml.tile([128, 6, NCH], bf16, tag="prod6")
            nc.vector.tensor_tensor(prod6[:], vqmn,
                                    e[:, None, 0:NCH].to_broadcast((128, 6, NCH)),
                                    AO.mult)
            part6 = psml.tile([128, 6], f32, tag="part6")
            nc.vector.reduce_sum(part6[:], prod6[:], axis=mybir.AxisListType.X)
            st[p] += [awvs, awf2, part6]

        def stage_psr(p):
            part6 = st[p][6]
            pp = st[p][1]
            nc.tensor.matmul(pp[0:6, 171:172], part6[:], ones_col[:],
                             start=True, stop=True)

        def stage_b2(p):
            idx = (p % HPC) * B + p // HPC
            cds, pp, e, sg, awvs, awf2, part6 = st.pop(p)
            vt = cds[:, 1, :].rearrange("p (c d) -> p c d", d=128)
            psv = pp[:, 64:66]
            rvec2 = psml.tile([6, FD], bf16, tag="rvec")
            nc.vector.tensor_copy(rvec2[:], pp[0:6, 171:172].to_broadcast((6, FD)))

            # V matmuls: quant chunks + residual + low-rank + mn corrections,
            # all accumulated into psv[d, 0:2] (corrections apply to both cols)
            for c in range(NCH):
                nc.tensor.matmul(psv, vt[:, c, :], awvs[:, c, :],
                                 start=(c == 0), stop=False)
            nc.tensor.matmul(psv, vfull_sb[:, p, :], awf2[:], start=False, stop=False)
            nc.tensor.matmul(psv, vpm_sb[:, p, :], rvec2[:], start=False, stop=True)

            recip = psml.tile([128, 1], f32, tag="recip")
            nc.vector.reciprocal(recip[:], sg[:])
            nc.vector.tensor_scalar(woin[0:64, idx:idx + 1], psv[0:64, 0:1],
                                    recip[0:64, 0:1], None, AO.mult)
            nc.vector.tensor_scalar(woin[64:128, idx:idx + 1], psv[64:128, 1:2],
                                    recip[64:128, 0:1], None, AO.mult)

        for r in range(NP + 2):
            if r >= 2:
                stage_b1(r - 2)
            if r < NP:
                stage_a(r)
            if r >= 2:
                stage_psr(r - 2)
            if r >= 1 and r - 1 < NP:
                stage_m(r - 1)
            if r >= 2:
                stage_b2(r - 2)

        # ---- tail: wo matmul ----
        ictx.close()
        psO = ctx.enter_context(tc.tile_pool(name="psO", bufs=2, space="PSUM"))
        for half in range(2):
            po = psO.tile([B, HID // 2], f32, tag="po")
            for h in range(HPC):
                for nb in range(4):
                    j0 = half * 2048 + nb * 512
                    nc.tensor.matmul(po[:, nb * 512:(nb + 1) * 512],
                                     woin[:, h * B:(h + 1) * B], wo_sb[:, h, j0:j0 + 512],
                                     start=(h == 0), stop=(h == HPC - 1))
            osb = const.tile([B, HID // 2], f32, tag=f"osb{half}")
            nc.scalar.copy(osb[:], po[:])
            nc.sync.dma_start(out=out[:, half * 2048:(half + 1) * 2048], in_=osb[:])

    nc.compile()
    return nc


def _host_prep(inputs):
    hs = np.asarray(inputs["hidden_states"], np.float32)
    pos = np.asarray(inputs["position_ids"])
    inv = 1.0 / (THETA ** (np.arange(0, D, 2, dtype=np.float32) / D))
    fr = pos[:, 0].astype(np.float32)[:, None] * inv[None, :]
    emb = np.concatenate([fr, fr], axis=1)
    cos_b = np.cos(emb).astype(np.float32)
    sin_b = np.sin(emb).astype(np.float32)
    cosin = np.ascontiguousarray(
        np.concatenate([np.tile(cos_b, (1, HPC)), np.tile(sin_b, (1, HPC))], axis=1))
    # hidq [128, NCH*B] bf16
    hidq = np.ascontiguousarray(
        hs[:, 0, :].T.reshape(NCH, 128, B).transpose(1, 0, 2).reshape(128, NCH * B)
    ).astype(BF16)

    wq, wk, wv, wo = (np.asarray(inputs[k], np.float32) for k in ("wq", "wk", "wv", "wo"))
    kq_f = np.asarray(inputs["k_quant"])
    vq_f = np.asarray(inputs["v_quant"])
    ks_f = np.asarray(inputs["k_scale"], np.float32)
    km_f = np.asarray(inputs["k_mn"], np.float32)
    kf_f = np.asarray(inputs["k_full"], np.float32)
    kp_f = np.asarray(inputs["key_p"], np.float32)
    kqr_f = np.asarray(inputs["key_q"], np.float32)
    vs_f = np.asarray(inputs["v_scale"], np.float32)
    vm_f = np.asarray(inputs["v_mn"], np.float32)
    vf_f = np.asarray(inputs["v_full"], np.float32)
    vqv_f = np.asarray(inputs["value_q"], np.float32)
    vp_f = np.asarray(inputs["value_p"], np.float32)

    in_maps = []
    for core in range(NCORES):
        h0 = core * HPC
        sl = slice(h0 * D, (h0 + HPC) * D)
        hsl = slice(h0, h0 + HPC)

        wqkv = np.concatenate([wq[sl].T, wk[sl].T, wv[sl].T], axis=1)  # [4096, 1536]
        wqkv = wqkv.reshape(NCH, 128, 3 * HPC * D).transpose(1, 0, 2).reshape(128, -1)
        woc = wo[:, sl].T.reshape(HPC, 128, HID).transpose(1, 0, 2).reshape(128, -1)

        codes = np.empty((NP, 128, 2 * SQ), FP8)
        codes[:, :, 0:SQ] = kq_f[:, hsl].reshape(NP, 128, SQ).astype(FP8)
        # vcode: [B,HPC,SQ,D] -> [pair, p=s%128, c, d]
        codes[:, :, SQ:2 * SQ] = (vq_f[:, hsl].reshape(B, HPC, NCH, 128, D)
                                  .transpose(0, 1, 3, 2, 4).reshape(NP, 128, NCH * 128)
                                  .astype(FP8))

        kblob = np.zeros((128, NP, KBW), np.float32)
        kblob[:, :, 0:G] = ks_f[:, hsl].reshape(NP, 128, G).transpose(1, 0, 2)
        kblob[:, :, G:2 * G] = km_f[:, hsl].reshape(NP, 128, G).transpose(1, 0, 2)
        kblob[:, :, 128:132] = kqr_f[:, hsl].reshape(NP, 128, RANK).transpose(1, 0, 2)
        kblob[:, :, 132:132 + SF] = kf_f[:, hsl].reshape(NP, SF, 128).transpose(2, 0, 1)
        kblob[:, :, 196:196 + NCH * RANK] = (
            kp_f[:, hsl].reshape(B, HPC, NCH, 128, RANK)
            .transpose(3, 0, 1, 2, 4).reshape(128, NP, NCH * RANK))

        vblob = np.zeros((128, NP, VBW), np.float32)
        vblob[:, :, 0:NCH * FD] = (
            vs_f[:, hsl].reshape(B, HPC, NCH, 128, FD)
            .transpose(3, 0, 1, 2, 4).reshape(128, NP, NCH * FD))
        vblob[:, :, NCH * FD:NCH * FD + 4 * NCH] = (
            vqv_f[:, hsl].reshape(B, HPC, NCH, 128, RANK)
            .transpose(3, 0, 1, 4, 2).reshape(128, NP, 4 * NCH))
        vblob[:, :, NCH * FD + 4 * NCH:VBW] = (
            vm_f[:, hsl].reshape(B, HPC, NCH, 128, FD)
            .transpose(3, 0, 1, 4, 2).reshape(128, NP, 2 * NCH))

        vpm = np.zeros((6, NP, 128), np.float32)
        vpm[0:4] = vp_f[:, hsl].reshape(NP, 128, RANK).transpose(2, 0, 1)
        vpm[4, :, 0:64] = 1.0
        vpm[5, :, 64:128] = 1.0

        vfull = vf_f[:, hsl].reshape(NP, SF, 128).transpose(1, 0, 2)

        m = {
            "hidq": hidq, "cosin": cosin,
            "wqkv": np.ascontiguousarray(wqkv).astype(BF16),
            "woc": np.ascontiguousarray(woc).astype(BF16),
            "codes": codes,
            "kblob": kblob.reshape(128, NP * KBW).astype(BF16),
            "vblob": vblob.reshape(128, NP * VBW).astype(BF16),
            "vpm": vpm.reshape(6, NP * 128).astype(BF16),
            "vfull": np.ascontiguousarray(vfull).reshape(SF, NP * 128).astype(BF16),
        }
        in_maps.append(m)
    return in_maps


def kernel(**inputs):
    if "nc" not in _CACHE:
        _CACHE["nc"] = _build()
    nc = _CACHE["nc"]
    in_maps = _host_prep(inputs)
    res = run_bass_kernel_spmd(nc, in_maps, list(range(NCORES)),
                               trace=bool(os.environ.get("K_TRACE")))
    kernel.last = res
    total = np.zeros((B, HID), np.float32)
    for r in res.results:
        total += r["out"]
    return total.reshape(B, QL, HID)


# revision 38
# speedup vs baseline: 2.2790x; 1.0233x over previous
"""GEAR quantized-KV Llama attention decode step on 8 trn2 NeuronCores.

Sharding: tensor-parallel over heads (4 heads/core x 8 cores), all batches on
every core; each core computes a partial wo-product, summed on host.

v2: fp8 codes (exact for 0..15), bf16 weights, packed per-core blobs so every
DMA is large and contiguous; K-score matmuls use fp8 FWL stationary codes;
V matmuls keep head-dim on partitions; broadcasts via ones-matmul.
"""
import os
import sys
import math

sys.path.insert(0, "/opt/trn_rl_repo")
import numpy as np
import ml_dtypes
from contextlib import ExitStack

import concourse.bass as bass
import concourse.mybir as mybir
import concourse.tile as tile
from concourse import bacc, bass_isa
from concourse.bass_utils import run_bass_kernel_spmd
from concourse.masks import make_identity

B, H, D, HID = 4, 32, 128, 4096
SQ, SF, QL = 4096, 63, 1
GS, RANK = 64, 4
THETA = 10000.0
NCORES = 8
HPC = H // NCORES          # heads per core = 4
NP = B * HPC               # (b,h) pairs per core = 16
NCH = SQ // 128            # 32 s-chunks
G = SQ // GS               # 64 groups along seq (K side)
FD = D // GS               # 2 groups along head_dim (V side)
SFP = SF + 1               # 64 full-precision keys incl the new token
DT = mybir.dt
ISQD = 1.0 / math.sqrt(D)
KBW = G + G + 64 + NCH * RANK + RANK   # kblob width = 64+64+64+128+4 = 324
VBW = NCH * FD + 6 * NCH               # vblob width = 64+192 = 256

BF16 = ml_dtypes.bfloat16
FP8 = ml_dtypes.float8_e4m3

_CACHE = {}


def _build():
    nc = bacc.Bacc("TRN2", target_bir_lowering=False)
    f32, bf16, fp8 = DT.float32, DT.bfloat16, DT.float8e4

    hidq = nc.declare_dram_parameter("hidq", [128, NCH * B], bf16, isOutput=False)
    cosin = nc.declare_dram_parameter("cosin", [B, 2 * HPC * D], f32, isOutput=False)
    wqkv = nc.declare_dram_parameter("wqkv", [128, NCH * 3 * HPC * D], bf16, isOutput=False)
    woc = nc.declare_dram_parameter("woc", [128, HPC * HID], bf16, isOutput=False)
    codes = nc.declare_dram_parameter("codes", [NP, 128, 2 * SQ], fp8, isOutput=False)
    kblob = nc.declare_dram_parameter("kblob", [128, NP * KBW], bf16, isOutput=False)
    vblob = nc.declare_dram_parameter("vblob", [128, NP * VBW], bf16, isOutput=False)
    vpm = nc.declare_dram_parameter("vpm", [6, NP * 128], bf16, isOutput=False)
    vfull = nc.declare_dram_parameter("vfull", [SF, NP * 128], bf16, isOutput=False)
    out = nc.declare_dram_parameter("out", [B, HID], f32, isOutput=True)

    AO = mybir.AluOpType
    AF = mybir.ActivationFunctionType

    with tile.TileContext(nc) as tc, ExitStack() as ctx:
        const = ctx.enter_context(tc.tile_pool(name="const", bufs=1))
        pw = ctx.enter_context(tc.tile_pool(name="pw", bufs=2))
        ictx = ctx.enter_context(ExitStack())
        psml = ictx.enter_context(tc.tile_pool(name="psml", bufs=4))
        pkc = ictx.enter_context(tc.tile_pool(name="pkc", bufs=4))
        psA = ictx.enter_context(tc.tile_pool(name="psA", bufs=4, space="PSUM"))
        pctx = ExitStack()
        psC = pctx.enter_context(tc.tile_pool(name="psC", bufs=1, space="PSUM"))

        # ---- constants / resident blobs ----
        id4 = const.tile([4, 4], f32)
        make_identity(nc, id4[:])
        ones_row = const.tile([1, 128], bf16)
        nc.vector.memset(ones_row[:], 1.0)
        ones_col = const.tile([128, 1], f32)
        nc.vector.memset(ones_col[:], 1.0)

        hid_sb = const.tile([128, NCH, B], bf16)
        nc.sync.dma_start(out=hid_sb[:], in_=hidq[:].rearrange("p (c b) -> p c b", b=B))
        cos_sb = const.tile([B, HPC * D], f32)
        nc.sync.dma_start(out=cos_sb[:], in_=cosin[:, 0:HPC * D])
        sin_sb = const.tile([B, HPC * D], f32)
        nc.sync.dma_start(out=sin_sb[:], in_=cosin[:, HPC * D:2 * HPC * D])
        kblob_sb = const.tile([128, NP, KBW], bf16)
        nc.sync.dma_start(out=kblob_sb[:], in_=kblob[:].rearrange("p (n w) -> p n w", w=KBW))
        vblob_sb = const.tile([128, NP, VBW], bf16)
        nc.sync.dma_start(out=vblob_sb[:], in_=vblob[:].rearrange("p (n w) -> p n w", w=VBW))
        vpm_sb = const.tile([6, NP, 128], bf16)
        nc.sync.dma_start(out=vpm_sb[:], in_=vpm[:].rearrange("p (n d) -> p n d", d=128))
        vfull_sb = const.tile([SFP, NP, 128], bf16)
        nc.sync.dma_start(out=vfull_sb[0:SF], in_=vfull[:].rearrange("p (n d) -> p n d", d=128))
        wo_sb = const.tile([128, HPC, HID], bf16)

        # ---- projections: psum[b, 1536] = sum_c hid_c^T @ wqkv_c ----
        pps = psC.tile([B, 3 * HPC * D], f32, tag="proj")
        for blk in range(4):
            slab = pw.tile([128, 8, 3 * HPC * D], bf16, tag="wslab")
            nc.scalar.dma_start(
                out=slab[:],
                in_=wqkv[:, 8 * blk * 1536:(8 * blk + 8) * 1536].rearrange(
                    "p (c n) -> p c n", n=1536),
            )
            for j in range(8):
                c = 8 * blk + j
                for nb in range(3):
                    nc.tensor.matmul(pps[:, nb * 512:(nb + 1) * 512],
                                     hid_sb[:, c, :], slab[:, j, nb * 512:(nb + 1) * 512],
                                     start=(c == 0), stop=(c == 31))
        qkv_sb = const.tile([B, 3 * HPC * D], f32)
        nc.scalar.copy(qkv_sb[:], pps[:])
        q_sb = qkv_sb[:, 0:512]
        k_sb = qkv_sb[:, 512:1024]

        # ---- RoPE on q and k (rows [B, HPC*D], f32) ----
        def rope(x_v, tagp):
            rot = const.tile([B, HPC * D], f32, tag=tagp + "rot")
            xv = x_v.rearrange("b (h two d) -> b h two d", two=2, d=64)
            rv = rot[:].rearrange("b (h two d) -> b h two d", two=2, d=64)
            nc.vector.tensor_scalar(rv[:, :, 0, :], xv[:, :, 1, :], -1.0, None, AO.mult)
            nc.vector.tensor_copy(rv[:, :, 1, :], xv[:, :, 0, :])
            nc.vector.tensor_tensor(rot[:], rot[:], sin_sb[:], AO.mult)
            ro = const.tile([B, HPC * D], f32, tag=tagp + "ro")
            nc.vector.tensor_tensor(ro[:], x_v, cos_sb[:], AO.mult)
            nc.vector.tensor_tensor(ro[:], ro[:], rot[:], AO.add)
            return ro
        qro = rope(q_sb, "q")
        kro = rope(k_sb, "k")

        # per-head transposed columns: qscT [128, h, b] (scaled by 1/sqrt(D)), kT
        qscT = const.tile([128, HPC, B], bf16)
        qsc32p = const.tile([128, NP], f32)       # pair-major (b*HPC+h) f32 copy
        qsc32v = qsc32p[:].rearrange("p (b h) -> p h b", h=HPC)
        kT = const.tile([128, HPC, B], bf16)
        for h in range(HPC):
            pq = psC.tile([128, B], f32, tag="tr")
            nc.tensor.transpose(pq[:], qro[0:B, h * D:(h + 1) * D], id4[:])
            nc.scalar.mul(qscT[:, h, :], pq[:], ISQD)
            nc.scalar.mul(qsc32v[:, h, :], pq[:], ISQD)
            pk = psC.tile([128, B], f32, tag="tr")
            nc.tensor.transpose(pk[:], kro[0:B, h * D:(h + 1) * D], id4[:])
            nc.scalar.copy(kT[:, h, :], pk[:])
        pctx.close()

        # new-token k columns for all pairs (one strided copy), v rows (cast DMAs
        # issued here so they run on the idle gpsimd queue during projections)
        nc.vector.tensor_copy(
            kblob_sb[:, :, 195:196].rearrange("p (b h) one -> p b (h one)", h=HPC),
            kT[:].rearrange("p h b -> p b h"))
        for p in range(NP):
            b, h = p // HPC, p % HPC
            nc.gpsimd.dma_start(
                out=vfull_sb[SF:SFP, p, :],
                in_=qkv_sb[b:b + 1, 1024 + h * D:1024 + (h + 1) * D])
        # qs for all pairs in one op: qs_all[p, pair, g] = kscale * q / sqrt(D)
        qs_all = const.tile([128, NP, G], bf16)
        nc.vector.tensor_tensor(qs_all[:], kblob_sb[:, :, 0:G],
                                qsc32p[:, :, None].to_broadcast((128, NP, G)), AO.mult)

        woin = const.tile([128, NP], bf16)

        # ---- per (b, h) attention, software-pipelined ----
        # Round r emits: B1(r-2) softmax-dependent DVE/gpsimd prep, A(r) DMA +
        # K-score matmuls + logit assembly, psr(r-2) matmul, M(r-1) exp chain,
        # B2(r-2) V matmuls + normalize + combine. Normalization is deferred to
        # the woin copies so nothing upstream waits on the exp-sum reduction.
        # pp layout: psk 0:64 | psv 64:66 | att 66:99 | qrow 99:167 |
        #            qrb 167:171 | psr 171:172
        st = {}

        def stage_a(p):
            b, h = p // HPC, p % HPC
            qcol = qscT[:, h, b:b + 1]
            if p == 4:
                # wo weights stream during attention, on the ACT ring
                nc.scalar.dma_start(out=wo_sb[:],
                                    in_=woc[:].rearrange("p (h n) -> p h n", n=HID))

            cds = pkc.tile([128, 2, SQ], fp8, tag="codes")
            nc.sync.dma_start(out=cds[:], in_=codes[p].rearrange(
                "p (two s) -> p two s", s=SQ))
            kc = cds[:, 0, :]

            kmnq = kblob_sb[:, p, G:G + 68]            # kmn | keyq
            kfp = kblob_sb[:, p, 132:132 + SFP]
            kp_v = kblob_sb[:, p, 196:196 + NCH * RANK].rearrange(
                "p (c r) -> p c r", r=RANK)

            pp = psA.tile([128, 172], f32, tag="pp")
            psk = pp[:, 0:64]

            # kf scores straight into the logit column [64, 98:99]
            nc.tensor.matmul(pp[0:SFP, 98:99], kfp, qcol, start=True, stop=True)
            # mn bias row [1, 99:163], qr row [1, 163:167]
            nc.tensor.matmul(pp[0:1, 99:167], qcol, kmnq, start=True, stop=True)
            qb_row = psml.tile([1, 68], bf16, tag="qbrow")
            nc.scalar.copy(qb_row[:], pp[0:1, 99:167])
            # bias broadcast seeds psk; chunk matmuls accumulate onto it.
            # qr broadcast to [128, 167:171]
            nc.tensor.matmul(psk, ones_row[:], qb_row[0:1, 0:64],
                             start=True, stop=False, skip_group_check=True)
            nc.tensor.matmul(pp[:, 167:171], ones_row[:], qb_row[0:1, 64:68],
                             start=True, stop=True)

            # quant K scores: psk[s, 2c + g'] += codes^T qs per chunk
            for c in range(NCH):
                nc.tensor.matmul(psk[:, 2 * c:2 * c + 2], kc[:, c * 128:(c + 1) * 128],
                                 qs_all[:, p, 2 * c:2 * c + 2], start=False,
                                 stop=(c == NCH - 1), skip_group_check=True)

            # low-rank correction lr[s, c] = sum_r kp[s,c,r] * qr[r]
            lrt = psml.tile([128, NCH, RANK], bf16, tag="lrt")
            nc.vector.tensor_tensor(lrt[:], kp_v,
                                    pp[:, None, 167:171].to_broadcast((128, NCH, RANK)),
                                    AO.mult)
            lr = psml.tile([128, NCH], f32, tag="lr")
            nc.vector.reduce_sum(lr[:], lrt[:], axis=mybir.AxisListType.X)

            # logits att = pp[:, 66:99]: quant scores + lr, kf col, -inf filler
            pskv = psk.rearrange("p (c two) -> p c two", two=2)
            nc.vector.tensor_tensor(pp[0:64, 66:98], pskv[0:64, :, 0], lr[0:64, :], AO.add)
            nc.vector.tensor_tensor(pp[64:128, 66:98], pskv[64:128, :, 1], lr[64:128, :], AO.add)
            nc.vector.memset(pp[64:128, 98:99], -1e9)

            m1 = psml.tile([128, 1], f32, tag="m1")
            nc.vector.reduce_max(m1[:], pp[:, 66:99], axis=mybir.AxisListType.X)
            mg = psml.tile([128, 1], f32, tag="mg")
            nc.gpsimd.partition_all_reduce(mg[:], m1[:], 128, bass_isa.ReduceOp.max)
            st[p] = [cds, pp, mg]

        def stage_m(p):
            cds, pp, mg = st[p]
            negm = psml.tile([128, 1], f32, tag="negm")
            nc.vector.tensor_scalar(negm[:], mg[:], -1.0, None, AO.mult)
            e = psml.tile([128, NCH + 1], bf16, tag="e")
            ssum = psml.tile([128, 1], f32, tag="ssum")
            nc.scalar.activation(e[:], pp[:, 66:99], AF.Exp, bias=negm[:, 0:1],
                                 scale=1.0, alpha=0.0, accum_out=ssum[:])
            sg = psml.tile([128, 1], f32, tag="sg")
            nc.gpsimd.partition_all_reduce(sg[:], ssum[:], 128, bass_isa.ReduceOp.add)
            st[p] = [cds, pp, e, sg]

        def stage_b1(p):
            cds, pp, e, sg = st[p]
            vsc = vblob_sb[:, p, 0:NCH * FD].rearrange("p (c g) -> p c g", g=FD)
            vqmn = vblob_sb[:, p, NCH * FD:VBW].rearrange("p (j c) -> p j c", c=NCH)

            # unnormalized moving cols: e * vscale per chunk (DVE)
            ev = e[:, 0:NCH, None]
            awvs = psml.tile([128, NCH, FD], bf16, tag="awvs")
            nc.vector.tensor_tensor(awvs[:], ev.to_broadcast((128, NCH, FD)), vsc,
                                    AO.mult)
            awf2 = psml.tile([SFP, FD], bf16, tag="awf")
            nc.vector.tensor_copy(awf2[:], e[0:SFP, NCH:NCH + 1].to_broadcast((SFP, FD)))
            prod6 = psml.tile([128, 6, NCH], bf16, tag="prod6")
            nc.vector.tensor_tensor(prod6[:], vqmn,
                                    e[:, None, 0:NCH].to_broadcast((128, 6, NCH)),
                                    AO.mult)
            part6 = psml.tile([128, 6], f32, tag="part6")
            nc.vector.reduce_sum(part6[:], prod6[:], axis=mybir.AxisListType.X)
            st[p] += [awvs, awf2, part6]

        def stage_psr(p):
            part6 = st[p][6]
            pp = st[p][1]
            nc.tensor.matmul(pp[0:6, 171:172], part6[:], ones_col[:],
                             start=True, stop=True)

        def stage_b2(p):
            cds, pp, e, sg, awvs, awf2, part6 = st[p]
            vt = cds[:, 1, :].rearrange("p (c d) -> p c d", d=128)
            psv = pp[:, 64:66]
            rvec2 = psml.tile([6, FD], bf16, tag="rvec")
            nc.vector.tensor_copy(rvec2[:], pp[0:6, 171:172].to_broadcast((6, FD)))

            # V matmuls: quant chunks + residual + low-rank + mn corrections,
            # all accumulated into psv[d, 0:2] (corrections apply to both cols)
            for c in range(NCH):
                nc.tensor.matmul(psv, vt[:, c, :], awvs[:, c, :],
                                 start=(c == 0), stop=False)
            nc.tensor.matmul(psv, vfull_sb[:, p, :], awf2[:], start=False, stop=False)
            nc.tensor.matmul(psv, vpm_sb[:, p, :], rvec2[:], start=False, stop=True)

        def stage_c(p):
            idx = (p % HPC) * B + p // HPC
            cds, pp, e, sg, awvs, awf2, part6 = st.pop(p)
            psv = pp[:, 64:66]
            recip = psml.tile([128, 1], f32, tag="recip")
            nc.vector.reciprocal(recip[:], sg[:])
            nc.vector.tensor_scalar(woin[0:64, idx:idx + 1], psv[0:64, 0:1],
                                    recip[0:64, 0:1], None, AO.mult)
            nc.vector.tensor_scalar(woin[64:128, idx:idx + 1], psv[64:128, 1:2],
                                    recip[64:128, 0:1], None, AO.mult)

        for r in range(NP + 3):
            if 1 <= r <= NP:
                stage_m(r - 1)
            if 2 <= r <= NP + 1:
                stage_b1(r - 2)
            if r < NP:
                stage_a(r)
            if 2 <= r <= NP + 1:
                stage_psr(r - 2)
                stage_b2(r - 2)
            if r >= 3:
                stage_c(r - 3)

        # ---- tail: wo matmul ----
        ictx.close()
        psO = ctx.enter_context(tc.tile_pool(name="psO", bufs=2, space="PSUM"))
        for half in range(2):
            po = psO.tile([B, HID // 2], f32, tag="po")
            for h in range(HPC):
                for nb in range(4):
                    j0 = half * 2048 + nb * 512
                    nc.tensor.matmul(po[:, nb * 512:(nb + 1) * 512],
                                     woin[:, h * B:(h + 1) * B], wo_sb[:, h, j0:j0 + 512],
                                     start=(h == 0), stop=(h == HPC - 1))
            osb = const.tile([B, HID // 2], f32, tag=f"osb{half}")
            nc.scalar.copy(osb[:], po[:])
            nc.sync.dma_start(out=out[:, half * 2048:(half + 1) * 2048], in_=osb[:])

    nc.compile()
    return nc


def _host_prep(inputs):
    hs = np.asarray(inputs["hidden_states"], np.float32)
    pos = np.asarray(inputs["position_ids"])
    inv = 1.0 / (THETA ** (np.arange(0, D, 2, dtype=np.float32) / D))
    fr = pos[:, 0].astype(np.float32)[:, None] * inv[None, :]
    emb = np.concatenate([fr, fr], axis=1)
    cos_b = np.cos(emb).astype(np.float32)
    sin_b = np.sin(emb).astype(np.float32)
    cosin = np.ascontiguousarray(
        np.concatenate([np.tile(cos_b, (1, HPC)), np.tile(sin_b, (1, HPC))], axis=1))
    # hidq [128, NCH*B] bf16
    hidq = np.ascontiguousarray(
        hs[:, 0, :].T.reshape(NCH, 128, B).transpose(1, 0, 2).reshape(128, NCH * B)
    ).astype(BF16)

    wq, wk, wv, wo = (np.asarray(inputs[k], np.float32) for k in ("wq", "wk", "wv", "wo"))
    kq_f = np.asarray(inputs["k_quant"])
    vq_f = np.asarray(inputs["v_quant"])
    ks_f = np.asarray(inputs["k_scale"], np.float32)
    km_f = np.asarray(inputs["k_mn"], np.float32)
    kf_f = np.asarray(inputs["k_full"], np.float32)
    kp_f = np.asarray(inputs["key_p"], np.float32)
    kqr_f = np.asarray(inputs["key_q"], np.float32)
    vs_f = np.asarray(inputs["v_scale"], np.float32)
    vm_f = np.asarray(inputs["v_mn"], np.float32)
    vf_f = np.asarray(inputs["v_full"], np.float32)
    vqv_f = np.asarray(inputs["value_q"], np.float32)
    vp_f = np.asarray(inputs["value_p"], np.float32)

    in_maps = []
    for core in range(NCORES):
        h0 = core * HPC
        sl = slice(h0 * D, (h0 + HPC) * D)
        hsl = slice(h0, h0 + HPC)

        wqkv = np.concatenate([wq[sl].T, wk[sl].T, wv[sl].T], axis=1)  # [4096, 1536]
        wqkv = wqkv.reshape(NCH, 128, 3 * HPC * D).transpose(1, 0, 2).reshape(128, -1)
        woc = wo[:, sl].T.reshape(HPC, 128, HID).transpose(1, 0, 2).reshape(128, -1)

        codes = np.empty((NP, 128, 2 * SQ), FP8)
        codes[:, :, 0:SQ] = kq_f[:, hsl].reshape(NP, 128, SQ).astype(FP8)
        # vcode: [B,HPC,SQ,D] -> [pair, p=s%128, c, d]
        codes[:, :, SQ:2 * SQ] = (vq_f[:, hsl].reshape(B, HPC, NCH, 128, D)
                                  .transpose(0, 1, 3, 2, 4).reshape(NP, 128, NCH * 128)
                                  .astype(FP8))

        kblob = np.zeros((128, NP, KBW), np.float32)
        kblob[:, :, 0:G] = ks_f[:, hsl].reshape(NP, 128, G).transpose(1, 0, 2)
        kblob[:, :, G:2 * G] = km_f[:, hsl].reshape(NP, 128, G).transpose(1, 0, 2)
        kblob[:, :, 128:132] = kqr_f[:, hsl].reshape(NP, 128, RANK).transpose(1, 0, 2)
        kblob[:, :, 132:132 + SF] = kf_f[:, hsl].reshape(NP, SF, 128).transpose(2, 0, 1)
        kblob[:, :, 196:196 + NCH * RANK] = (
            kp_f[:, hsl].reshape(B, HPC, NCH, 128, RANK)
            .transpose(3, 0, 1, 2, 4).reshape(128, NP, NCH * RANK))

        vblob = np.zeros((128, NP, VBW), np.float32)
        vblob[:, :, 0:NCH * FD] = (
            vs_f[:, hsl].reshape(B, HPC, NCH, 128, FD)
            .transpose(3, 0, 1, 2, 4).reshape(128, NP, NCH * FD))
        vblob[:, :, NCH * FD:NCH * FD + 4 * NCH] = (
            vqv_f[:, hsl].reshape(B, HPC, NCH, 128, RANK)
            .transpose(3, 0, 1, 4, 2).reshape(128, NP, 4 * NCH))
        vblob[:, :, NCH * FD + 4 * NCH:VBW] = (
            vm_f[:, hsl].reshape(B, HPC, NCH, 128, FD)
            .transpose(3, 0, 1, 4, 2).reshape(128, NP, 2 * NCH))

        vpm = np.zeros((6, NP, 128), np.float32)
        vpm[0:4] = vp_f[:, hsl].reshape(NP, 128, RANK).transpose(2, 0, 1)
        vpm[4, :, 0:64] = 1.0
        vpm[5, :, 64:128] = 1.0

        vfull = vf_f[:, hsl].reshape(NP, SF, 128).transpose(1, 0, 2)

        m = {
            "hidq": hidq, "cosin": cosin,
            "wqkv": np.ascontiguousarray(wqkv).astype(BF16),
            "woc": np.ascontiguousarray(woc).astype(BF16),
            "codes": codes,
            "kblob": kblob.reshape(128, NP * KBW).astype(BF16),
            "vblob": vblob.reshape(128, NP * VBW).astype(BF16),
            "vpm": vpm.reshape(6, NP * 128).astype(BF16),
            "vfull": np.ascontiguousarray(vfull).reshape(SF, NP * 128).astype(BF16),
        }
        in_maps.append(m)
    return in_maps


def kernel(**inputs):
    if "nc" not in _CACHE:
        _CACHE["nc"] = _build()
    nc = _CACHE["nc"]
    in_maps = _host_prep(inputs)
    res = run_bass_kernel_spmd(nc, in_maps, list(range(NCORES)),
                               trace=bool(os.environ.get("K_TRACE")))
    kernel.last = res
    total = np.zeros((B, HID), np.float32)
    for r in res.results:
        total += r["out"]
    return total.reshape(B, QL, HID)
